# revision 25
# baseline (speedup 1.0000x reference)
"""BERTgrid generator kernel for Trainium2 (8 NeuronCores, batch-parallel).

Per core (one document):
  emb [512, 768] f32, coors [512, 4] i32, mask [512, 1] i32
  -> out [768, 128*96] f16 grid (channel-major), host-cast to f32.

Device algorithm (no host compute on input values):
  1. valid/new_word/seg via triangular-matmul cumsums.
  2. Word mean table (shifted by one word) via one-hot matmul + reciprocal.
     Words span >=2 tokens, so word ids < 256 -> 2 chunks of 128.
  3. Band palettes: the grid is split into 4 bands of 32 rows; boxes span
     <=5 rows so each box touches <=2 bands. Words hitting a band get
     band-local ids by an inclusive cumsum (rank); <=128 words hit any
     band. A compact per-band table ctab[b] = Pw[b]^T @ table is built on
     the PE.
  4. Per-pixel last-covering band-local id via ONE exponent-weighted
     matmul: ps[r,c] = sum over covering words of 2^rank (per the row's
     band); ranks are distinct per band so the f32 exponent of the sum is
     exactly rank_max, i.e. widx = (bits >> 23) - 1 (-1 where uncovered).
  5. Paint: out[d, p] = ctab[band(p)][widx[p], d] via one-hot matmul in
     fp16, one 128-word chunk per 512-pixel slice (slices never straddle
     bands: 3072 % 512 == 0).
"""

import sys

import numpy as np

try:
    import concourse.bass as bass
except ImportError:  # grading env fallback
    sys.path.insert(0, "/opt/trn_rl_repo")
    import concourse.bass as bass

from concourse import bacc
import concourse.tile as tile
from concourse import mybir
from concourse.bass_utils import run_bass_kernel_spmd
from contextlib import ExitStack

P = 128
S, D = 512, 768
R, C, STRIDE = 128, 96, 8
T = S // P            # token tiles
NW = 256              # max word ids (each word spans >=2 tokens)
WT = NW // P          # word chunks for the global table (2)
NB = 4                # row bands (32 rows each)
BROWS = R // NB       # 32
BPIX = BROWS * C      # 3072 pixels per band
NPIX = R * C          # 12288
PG = 2048             # pixels per paint group
NG = NPIX // PG
DT = D // P
NH = PG // 512        # matmul column-slices per psum tile

F32 = mybir.dt.float32
F16 = mybir.dt.float16
BF16 = mybir.dt.bfloat16
I32 = mybir.dt.int32
OP = mybir.AluOpType

_last_results = None


def _const_blocks():
    """Host-precomputed constants, embedded in the NEFF (input-independent)."""
    p = np.arange(P, dtype=np.float32)[:, None]
    iota_r = np.broadcast_to(np.arange(R, dtype=np.float32), (P, R))
    iota_c = np.broadcast_to(np.arange(C, dtype=np.float32), (P, C))
    iota_w = np.broadcast_to(np.arange(NW, dtype=np.float32) - 1.0, (P, NW))
    iota_wp = np.broadcast_to(p, (P, 1)).copy()                 # palette ids
    bands_lo = np.broadcast_to(np.arange(NB, dtype=np.float32) * BROWS, (P, NB))
    bands_hi = bands_lo + BROWS
    rowband = np.broadcast_to(np.arange(R, dtype=np.float32) // BROWS, (P, R))
    cf32 = np.concatenate([iota_r, iota_c, iota_w, iota_wp,
                           bands_lo, bands_hi, rowband], axis=1)
    ii = np.arange(P)
    tri = (ii[:, None] <= ii[None, :]).astype(np.float16)  # [j, i] = j <= i
    ones = np.ones((P, P), dtype=np.float16)
    cf16 = np.concatenate([tri, ones], axis=1)
    return np.ascontiguousarray(cf32), np.ascontiguousarray(cf16)


def _build():
    nc = bacc.Bacc(None, target_bir_lowering=False)
    emb_ext = nc.declare_dram_parameter("emb", [S, D], F32, isOutput=False)
    coors_ext = nc.declare_dram_parameter("coors", [S, 4], I32, isOutput=False)
    mask_ext = nc.declare_dram_parameter("mask", [S, 1], I32, isOutput=False)
    out_ext = nc.declare_dram_parameter("out", [D, NPIX], F16, isOutput=True)
    widx_dram = nc.dram_tensor("widx_scratch", [P, C], F16)
    cf32_np, cf16_np = _const_blocks()
    cf32_ext = nc.inline_tensor(cf32_np, "cons_f32")
    cf16_ext = nc.inline_tensor(cf16_np, "cons_f16")

    with tile.TileContext(nc) as tc, ExitStack() as ctx:
        sing = ctx.enter_context(tc.tile_pool(name="sing", bufs=1))

        # ---- const + input loads, split across both HWDGE queues ----
        cf16 = sing.tile([P, 2 * P], F16, tag="cf16")
        nc.sync.dma_start(out=cf16[:], in_=cf16_ext[:])
        mask_all = sing.tile([P, T], I32, tag="mask_all")
        nc.scalar.dma_start(
            out=mask_all[:].rearrange("p (t o) -> p t o", t=T),
            in_=mask_ext[:].rearrange("(t p) o -> p t o", t=T))
        NC32 = R + C + NW + 1 + NB + NB + R
        cf32 = sing.tile([P, NC32], F32, tag="cf32")
        nc.scalar.dma_start(out=cf32[:], in_=cf32_ext[:])
        off = 0
        iota_r = cf32[:, off:off + R]; off += R
        iota_c = cf32[:, off:off + C]; off += C
        iotaW = cf32[:, off:off + NW]; off += NW
        iotawp = cf32[:, off:off + 1]; off += 1
        bandsLo = cf32[:, off:off + NB]; off += NB
        bandsHi = cf32[:, off:off + NB]; off += NB
        rowband = cf32[:, off:off + R]; off += R
        iota4 = iota_r[:, 0:NB]
        tri = cf16[:, 0:P]
        ones16 = cf16[:, P:2 * P]

        coors_all = sing.tile([P, 4 * T], I32, tag="coors_all")
        coors_re = coors_ext[:].rearrange("(t p) c -> p t c", t=T)
        nc.sync.dma_start(
            out=coors_all[:].rearrange("p (t c) -> p t c", t=T),
            in_=coors_re)
        coorsm1_all = sing.tile([P, 4 * T], I32, tag="coorsm1_all")
        nc.vector.memset(coorsm1_all[0:1, 0:4], -1)
        nc.sync.dma_start(out=coorsm1_all[1:P, 0:4],
                          in_=coors_ext[0:P - 1, :])
        nc.sync.dma_start(out=coorsm1_all[:, 4:8],
                          in_=coors_ext[P - 1:2 * P - 1, :])
        nc.scalar.dma_start(out=coorsm1_all[:, 8:12],
                            in_=coors_ext[2 * P - 1:3 * P - 1, :])
        nc.scalar.dma_start(out=coorsm1_all[:, 12:16],
                            in_=coors_ext[3 * P - 1:4 * P - 1, :])
        embT = sing.tile([P, T * (D + 1)], F16, tag="embT")
        embT3 = embT[:].rearrange("p (t e) -> p t e", e=D + 1)
        embext = [embT[:, t * (D + 1):(t + 1) * (D + 1)] for t in range(T)]

        # ---- batched per-token quantities ----
        mf = sing.tile([P, T], F32, tag="maskf")
        nc.vector.tensor_copy(mf[:], mask_all[:])
        invm4 = sing.tile([P, T], F16, tag="invm4")
        nc.vector.tensor_scalar(out=invm4[:], in0=mf[:], scalar1=1.0,
                                scalar2=-1.0, op0=OP.subtract, op1=OP.mult)
        cf = sing.tile([P, 4 * T], F32, tag="coorsf")
        nc.vector.tensor_copy(cf[:], coors_all[:])
        cm1f = sing.tile([P, 4 * T], F32, tag="coorsm1f")
        nc.vector.tensor_copy(cm1f[:], coorsm1_all[:])
        nc.gpsimd.memset(embT3[:, :, D:D + 1], 1.0)
        nc.vector.tensor_copy(
            embT3[:, :, 0:1],
            cm1f[:].rearrange("p (t c) -> p t c", t=T)[:, :, 0:1])
        nc.gpsimd.dma_start(
            out=embT3[:, :, 0:D],
            in_=emb_ext[:].rearrange("(t p) d -> p t d", t=T))
        eq16 = sing.tile([P, 4 * T], F32, tag="eq16")
        nc.vector.tensor_tensor(eq16[:], cf[:], cm1f[:], OP.is_equal)
        same4 = sing.tile([P, T], F32, tag="same4")
        nc.vector.tensor_reduce(same4[:],
                                eq16[:].rearrange("p (t c) -> p t c", t=T),
                                mybir.AxisListType.X, OP.min)
        wci = sing.tile([P, 4 * T], I32, tag="wci")
        nc.vector.tensor_scalar(out=wci[:], in0=coors_all[:], scalar1=3,
                                scalar2=None, op0=OP.arith_shift_right)
        wcf = sing.tile([P, 4 * T], F32, tag="wcf")
        nc.vector.tensor_copy(wcf[:], wci[:])
        wcf3 = wcf[:].rearrange("p (t c) -> p t c", t=T)
        wci3 = wci[:].rearrange("p (t c) -> p t c", t=T)

        # ---- valid / seg cumsums + per-band token ranks ----
        valid4 = sing.tile([P, T], F32, tag="valid4")
        nw4 = sing.tile([P, T], F32, tag="nw4")
        nwb4 = sing.tile([P, T], F16, tag="nwb4")
        seg4 = sing.tile([P, T], F32, tag="seg4")
        rank0 = sing.tile([P, T], F32, tag="rank0")
        rank1 = sing.tile([P, T], F32, tag="rank1")
        b0f = sing.tile([P, T], F32, tag="b0f")
        with tc.tile_pool(name="psA", bufs=1, space="PSUM") as psA:
            vps = psA.tile([P, T], F32, tag="vps", name="vps")
            for mt in range(T):
                for kc in range(mt + 1):
                    nc.tensor.matmul(out=vps[:, mt:mt + 1],
                                     lhsT=(tri if kc == mt else ones16),
                                     rhs=invm4[:, kc:kc + 1],
                                     start=(kc == 0), stop=(kc == mt))
            nc.vector.tensor_scalar(out=valid4[:], in0=vps[:],
                                    scalar1=0.5, scalar2=None, op0=OP.is_lt)
            nc.vector.scalar_tensor_tensor(out=nw4[:], in0=same4[:], scalar=0.5,
                                           in1=valid4[:], op0=OP.is_lt,
                                           op1=OP.mult)
            nc.vector.tensor_copy(nwb4[:], nw4[:])

            # coverage masks only need wcf -- keep DVE busy during cumsums
            rowcov, colcov = [], []
            for t in range(T):
                y0, y1 = wcf[:, 4 * t + 1:4 * t + 2], wcf[:, 4 * t + 3:4 * t + 4]
                x0, x1 = wcf[:, 4 * t + 0:4 * t + 1], wcf[:, 4 * t + 2:4 * t + 3]
                tge = sing.tile([P, R], F32, tag="tge")
                nc.vector.tensor_scalar(out=tge[:], in0=iota_r, scalar1=y0,
                                        scalar2=None, op0=OP.is_ge)
                rc = sing.tile([P, R], BF16, tag=f"rowcov{t}")
                nc.vector.scalar_tensor_tensor(out=rc[:], in0=iota_r, scalar=y1,
                                               in1=tge[:], op0=OP.is_lt,
                                               op1=OP.mult)
                rowcov.append(rc)
                cge = sing.tile([P, C], F32, tag="cge")
                nc.vector.tensor_scalar(out=cge[:], in0=iota_c, scalar1=x0,
                                        scalar2=None, op0=OP.is_ge)
                ccv = sing.tile([P, C], BF16, tag=f"colcov{t}")
                nc.vector.scalar_tensor_tensor(out=ccv[:], in0=iota_c, scalar=x1,
                                               in1=cge[:], op0=OP.is_lt,
                                               op1=OP.mult)
                colcov.append(ccv)

            sps = psA.tile([P, T], F32, tag="sps", name="sps")
            for mt in range(T):
                for kc in range(mt + 1):
                    nc.tensor.matmul(out=sps[:, mt:mt + 1],
                                     lhsT=(tri if kc == mt else ones16),
                                     rhs=nwb4[:, kc:kc + 1],
                                     start=(kc == 0), stop=(kc == mt))
            nc.vector.tensor_scalar(out=seg4[:], in0=sps[:],
                                    scalar1=1.0, scalar2=None, op0=OP.subtract)

            # per-token band hits: token box rows [y0g, y1g) vs 32-row bands
            y0gv = wcf3[:, :, 1:2]
            y1gv = wcf3[:, :, 3:4]
            hit = sing.tile([P, T * NB], F32, tag="hit")
            hitB = sing.tile([P, T * NB], F32, tag="hitB")
            hit3 = hit[:].rearrange("p (t b) -> p t b", t=T)
            hitB3 = hitB[:].rearrange("p (t b) -> p t b", t=T)
            nc.vector.tensor_tensor(hit3,
                                    y0gv.broadcast_to([P, T, NB]),
                                    bandsHi.unsqueeze(1).broadcast_to([P, T, NB]),
                                    OP.is_lt)
            nc.vector.tensor_tensor(hitB3,
                                    y1gv.broadcast_to([P, T, NB]),
                                    bandsLo.unsqueeze(1).broadcast_to([P, T, NB]),
                                    OP.is_gt)
            nc.vector.tensor_tensor(hit[:], hit[:], hitB[:], OP.mult)
            nwhit16 = sing.tile([P, T * NB], F16, tag="nwhit16")
            nc.vector.tensor_tensor(
                nwhit16[:].rearrange("p (t b) -> p t b", t=T),
                hit3, nw4[:].unsqueeze(2).broadcast_to([P, T, NB]), OP.mult)

            rps = psA.tile([P, T * NB], F32, tag="rps", name="rps")
            for mt in range(T):
                for kc in range(mt + 1):
                    nc.tensor.matmul(out=rps[:, mt * NB:(mt + 1) * NB],
                                     lhsT=(tri if kc == mt else ones16),
                                     rhs=nwhit16[:, kc * NB:(kc + 1) * NB],
                                     start=(kc == 0), stop=(kc == mt))

            # token's own band b0 = y0g >> 5 (box touches b0, maybe b0+1)
            b0i = sing.tile([P, T], I32, tag="b0i")
            nc.vector.tensor_scalar(
                out=b0i[:].rearrange("p (t o) -> p t o", o=1),
                in0=wci3[:, :, 1:2], scalar1=5,
                scalar2=None, op0=OP.arith_shift_right)
            nc.vector.tensor_copy(b0f[:], b0i[:])
            b0p1 = sing.tile([P, T], F32, tag="b0p1")
            nc.vector.tensor_scalar(out=b0p1[:], in0=b0f[:], scalar1=1.0,
                                    scalar2=None, op0=OP.add)
            oneh = sing.tile([P, T * NB], F32, tag="oneh")
            oneh3 = oneh[:].rearrange("p (t b) -> p t b", t=T)
            for bsel, rk in ((b0f, rank0), (b0p1, rank1)):
                nc.vector.tensor_tensor(
                    oneh3, iota4.unsqueeze(1).broadcast_to([P, T, NB]),
                    bsel[:].unsqueeze(2).broadcast_to([P, T, NB]), OP.is_equal)
                nc.vector.tensor_tensor(oneh[:], oneh[:], rps[:], OP.mult)
                nc.vector.tensor_reduce(rk[:], oneh3, mybir.AxisListType.X,
                                        OP.add)

        # scan weights: 2^rank per touched band (rank = pal_id + 1, distinct
        # within a band, so exponent of the pixel-sum = max rank)
        cw0 = sing.tile([P, T], F32, tag="cw0")
        cw1 = sing.tile([P, T], F32, tag="cw1")
        rbits = sing.tile([P, T], I32, tag="rbits")
        for rk, cw in ((rank0, cw0), (rank1, cw1)):
            nc.vector.tensor_copy(rbits[:], rk[:])
            nc.vector.tensor_scalar(out=rbits[:], in0=rbits[:], scalar1=23,
                                    scalar2=None, op0=OP.logical_shift_left)
            nc.vector.tensor_tensor(cw[:], rbits[:].bitcast(F32), nw4[:],
                                    OP.mult)
        rc0s, rc1s, rhs0s, rhs1s = [], [], [], []
        for t in range(T):
            rc0 = sing.tile([P, R], BF16, tag=f"rc0_{t}")
            nc.vector.scalar_tensor_tensor(out=rc0[:], in0=rowband,
                                           scalar=b0f[:, t:t + 1],
                                           in1=rowcov[t][:], op0=OP.is_equal,
                                           op1=OP.mult)
            rc1 = sing.tile([P, R], BF16, tag=f"rc1_{t}")
            nc.vector.tensor_tensor(rc1[:], rowcov[t][:], rc0[:], OP.subtract)
            r0 = sing.tile([P, C], BF16, tag=f"rhs0_{t}")
            nc.vector.tensor_scalar(out=r0[:], in0=colcov[t][:],
                                    scalar1=cw0[:, t:t + 1], scalar2=None,
                                    op0=OP.mult)
            r1 = sing.tile([P, C], BF16, tag=f"rhs1_{t}")
            nc.vector.tensor_scalar(out=r1[:], in0=colcov[t][:],
                                    scalar1=cw1[:, t:t + 1], scalar2=None,
                                    op0=OP.mult)
            rc0s.append(rc0); rc1s.append(rc1)
            rhs0s.append(r0); rhs1s.append(r1)

        # word-level one-hots: Onw for word boxes, Opr for the mean table
        seg4m1 = sing.tile([P, T], F32, tag="seg4m1")
        nc.vector.tensor_scalar(out=seg4m1[:], in0=seg4[:], scalar1=1.0,
                                scalar2=None, op0=OP.subtract)
        Onw, Opr = [], []
        for t in range(T):
            o = sing.tile([P, NW], F16, tag=f"onw{t}")
            nc.vector.tensor_scalar(out=o[:], in0=iotaW,
                                    scalar1=seg4m1[:, t:t + 1],
                                    scalar2=nw4[:, t:t + 1],
                                    op0=OP.is_equal, op1=OP.mult)
            Onw.append(o)
            o2 = sing.tile([P, NW], F16, tag=f"op{t}")
            nc.vector.tensor_scalar(out=o2[:], in0=iotaW,
                                    scalar1=seg4[:, t:t + 1],
                                    scalar2=valid4[:, t:t + 1],
                                    op0=OP.is_equal, op1=OP.mult)
            Opr.append(o2)
        ybox16 = sing.tile([P, T * 2], F16, tag="ybox16")
        ybox3 = ybox16[:].rearrange("p (t c) -> p t c", t=T)
        nc.vector.tensor_copy(ybox3[:, :, 0:1], wcf3[:, :, 1:2])
        nc.vector.tensor_copy(ybox3[:, :, 1:2], wcf3[:, :, 3:4])

        widx16 = sing.tile([P, C], F16, tag="widx16")
        widx_i = sing.tile([P, C], I32, tag="widx_i")
        table16 = []
        Pw = [[None] * WT for _ in range(NB)]
        with tc.tile_pool(name="psC", bufs=1, space="PSUM") as psC:
            # pixel scan: one accumulated stage over both touched bands
            ps1 = psC.tile([P, C], F32, tag="ps1")
            nmm = 2 * T
            k = 0
            for t in range(T):
                for rc, rh in ((rc0s[t], rhs0s[t]), (rc1s[t], rhs1s[t])):
                    nc.tensor.matmul(out=ps1[:], lhsT=rc[:], rhs=rh[:],
                                     start=(k == 0), stop=(k == nmm - 1))
                    k += 1
            nc.vector.tensor_scalar(out=widx_i[:], in0=ps1[:].bitcast(I32),
                                    scalar1=23, scalar2=None,
                                    op0=OP.logical_shift_right)
            nc.vector.tensor_scalar(out=widx_i[:], in0=widx_i[:], scalar1=1,
                                    scalar2=None, op0=OP.subtract)
            nc.vector.tensor_copy(widx16[:], widx_i[:])
            nc.sync.dma_start(out=widx_dram[:], in_=widx16[:])
            widx_flat = widx_dram[:].rearrange("p c -> (p c)")
            widx_g = []
            for g in range(NG):
                wg = sing.tile([P, PG], F16, tag=f"widx_g{g}")
                nc.sync.dma_start(
                    out=wg[:],
                    in_=widx_flat[g * PG:(g + 1) * PG].partition_broadcast(P))
                widx_g.append(wg)

            # word boxes: Wy[w, 2ch:2ch+2] = (y0g, y1g) of word w (chunk ch)
            Wy = psC.tile([P, 2 * WT], F32, tag="Wy")
            for ch in range(WT):
                for t in range(T):
                    nc.tensor.matmul(out=Wy[:, 2 * ch:2 * ch + 2],
                                     lhsT=Onw[t][:, ch * P:(ch + 1) * P],
                                     rhs=ybox16[:, 2 * t:2 * t + 2],
                                     start=(t == 0), stop=(t == T - 1))
            hitwf = sing.tile([P, WT * NB], F32, tag="hitwf")
            hitwB = sing.tile([P, WT * NB], F32, tag="hitwB")
            for ch in range(WT):
                sl = slice(ch * NB, (ch + 1) * NB)
                nc.vector.tensor_tensor(
                    hitwf[:, sl],
                    Wy[:, 2 * ch:2 * ch + 1].broadcast_to([P, NB]),
                    bandsHi, OP.is_lt)
                nc.vector.tensor_tensor(
                    hitwB[:, sl],
                    Wy[:, 2 * ch + 1:2 * ch + 2].broadcast_to([P, NB]),
                    bandsLo, OP.is_gt)
            nc.vector.tensor_tensor(hitwf[:], hitwf[:], hitwB[:], OP.mult)
            hitw16 = sing.tile([P, WT * NB], F16, tag="hitw16")
            nc.vector.tensor_copy(hitw16[:], hitwf[:])

            wrps = psC.tile([P, WT * NB], F32, tag="wrps")
            for mc in range(WT):
                for kc in range(mc + 1):
                    nc.tensor.matmul(out=wrps[:, mc * NB:(mc + 1) * NB],
                                     lhsT=(tri if kc == mc else ones16),
                                     rhs=hitw16[:, kc * NB:(kc + 1) * NB],
                                     start=(kc == 0), stop=(kc == mc))
            palwf = sing.tile([P, WT * NB], F32, tag="palwf")
            nc.vector.tensor_scalar(out=palwf[:], in0=wrps[:], scalar1=1.0,
                                    scalar2=None, op0=OP.subtract)
            for b in range(NB):
                for ch in range(WT):
                    pw = sing.tile([P, P], F16, tag=f"pw{b}_{ch}")
                    col = ch * NB + b
                    nc.vector.tensor_scalar(out=pw[:], in0=iota_r,
                                            scalar1=palwf[:, col:col + 1],
                                            scalar2=hitwf[:, col:col + 1],
                                            op0=OP.is_equal, op1=OP.mult)
                    Pw[b][ch] = pw

            # global word mean table (shifted): table[w] = mean(word w-1)
            with tc.tile_pool(name="psD", bufs=2, space="PSUM") as psD:
                for wt in range(WT):
                    ptab = psD.tile([P, 1024], F32, tag="ptab", name=f"ptab{wt}")
                    for kc in range(T):
                        lhs = Opr[kc][:, wt * P:(wt + 1) * P]
                        nc.tensor.matmul(out=ptab[:, 0:512], lhsT=lhs,
                                         rhs=embext[kc][:, 0:512],
                                         start=(kc == 0), stop=(kc == T - 1))
                        nc.tensor.matmul(out=ptab[:, 512:D + 1], lhsT=lhs,
                                         rhs=embext[kc][:, 512:D + 1],
                                         start=(kc == 0), stop=(kc == T - 1))
                    rec = sing.tile([P, 1], F32, tag="rec")
                    nc.vector.tensor_scalar(out=rec[:], in0=ptab[:, D:D + 1],
                                            scalar1=1.0, scalar2=None,
                                            op0=OP.max)
                    recr = sing.tile([P, 1], F32, tag="recr")
                    nc.vector.reciprocal(recr[:], rec[:])
                    tb = sing.tile([P, D], F16, tag=f"table{wt}")
                    nc.scalar.mul(out=tb[:], in_=ptab[:, 0:D], mul=recr[:, 0:1])
                    table16.append(tb)

        # compact per-band tables: ctab[b] = Pw[b]^T @ table
        ctab16 = []
        with tc.tile_pool(name="psE", bufs=2, space="PSUM") as psE:
            for b in range(NB):
                cps = psE.tile([P, D], F32, tag="cps", name=f"cps{b}")
                for ch in range(WT):
                    nc.tensor.matmul(out=cps[:, 0:512], lhsT=Pw[b][ch][:],
                                     rhs=table16[ch][:, 0:512],
                                     start=(ch == 0), stop=(ch == WT - 1))
                    nc.tensor.matmul(out=cps[:, 512:D], lhsT=Pw[b][ch][:],
                                     rhs=table16[ch][:, 512:D],
                                     start=(ch == 0), stop=(ch == WT - 1))
                ct = sing.tile([P, D], F16, tag=f"ctab{b}")
                if b % 2 == 0:
                    nc.vector.tensor_copy(ct[:], cps[:])
                else:
                    nc.scalar.copy(out=ct[:], in_=cps[:])
                ctab16.append(ct)

        # ---- paint: out[d, p] = ctab[band(p)][widx[p], d] ----
        with tc.tile_pool(name="oh", bufs=2) as ohp, \
             tc.tile_pool(name="stage", bufs=6) as stp, \
             tc.tile_pool(name="pp", bufs=2, space="PSUM") as ppp:
            for g in range(NG):
                gs = slice(g * PG, (g + 1) * PG)
                oh = ohp.tile([P, PG], F16, tag="oh", name="oh")
                nc.vector.tensor_scalar(out=oh[:], in0=widx_g[g][:],
                                        scalar1=iotawp[:, 0:1],
                                        scalar2=None, op0=OP.is_equal)
                for dt in range(DT):
                    stage = stp.tile([P, PG], F16, tag="stage", name="stage")
                    pp = ppp.tile([P, PG], F32, tag="pp", name="pp")
                    dsl = slice(dt * P, (dt + 1) * P)
                    for s3 in range(NH):
                        band = (g * PG + s3 * 512) // BPIX
                        nc.tensor.matmul(
                            out=pp[:, s3 * 512:(s3 + 1) * 512],
                            lhsT=ctab16[band][:, dsl],
                            rhs=oh[:, s3 * 512:(s3 + 1) * 512],
                            start=True, stop=True)
                    # PSUM->SBUF copies are PSUM-read bound (~2us full width):
                    # split each across both engines (different banks)
                    nc.vector.tensor_copy(stage[:, 0:PG // 2], pp[:, 0:PG // 2])
                    nc.scalar.copy(out=stage[:, PG // 2:PG],
                                   in_=pp[:, PG // 2:PG])
                    if dt % 2 == 0:
                        nc.sync.dma_start(out=out_ext[dt * P:(dt + 1) * P, gs],
                                          in_=stage[:])
                    else:
                        nc.scalar.dma_start(out=out_ext[dt * P:(dt + 1) * P, gs],
                                            in_=stage[:])
    nc.compile()
    return nc


_nc_cache = None


def kernel(bert_embeddings, coors, mask, image_h=1024, image_w=768, stride=8):
    global _last_results, _nc_cache
    emb = np.ascontiguousarray(np.asarray(bert_embeddings, dtype=np.float32))
    co = np.ascontiguousarray(np.asarray(coors, dtype=np.int32))
    mk = np.ascontiguousarray(np.asarray(mask, dtype=np.int32))
    ih, iw, st = int(image_h), int(image_w), int(stride)
    B = emb.shape[0]
    assert (ih // st, iw // st) == (R, C) and st == STRIDE
    assert emb.shape == (B, S, D) and B == 8

    if _nc_cache is None:
        _nc_cache = _build()
    nc = _nc_cache

    in_maps = [{"emb": emb[b], "coors": co[b], "mask": mk[b].reshape(S, 1)}
               for b in range(B)]
    res = run_bass_kernel_spmd(nc, in_maps, core_ids=list(range(B)))
    _last_results = res
    out = np.stack([np.asarray(res.results[b]["out"]).reshape(D, R, C)
                    for b in range(B)])
    return out.astype(np.float32)


# revision 26
# speedup vs baseline: 1.0730x; 1.0730x over previous
"""BERTgrid generator kernel for Trainium2 (8 NeuronCores, batch-parallel).

Per core (one document):
  emb [512, 768] f32, coors [512, 4] i32, mask [512, 1] i32
  -> out [768, 128*96] f16 grid (channel-major), host-cast to f32.

Device algorithm (no host compute on input values):
  1. valid/new_word/seg via triangular-matmul cumsums.
  2. Word mean table (shifted by one word) via one-hot matmul + reciprocal.
     Words span >=2 tokens, so word ids < 256 -> 2 chunks of 128.
  3. Band palettes: the grid is split into 4 bands of 32 rows; boxes span
     <=5 rows so each box touches <=2 bands. Words hitting a band get
     band-local ids by an inclusive cumsum (rank); <=128 words hit any
     band. A compact per-band table ctab[b] = Pw[b]^T @ table is built on
     the PE.
  4. Per-pixel last-covering band-local id via ONE exponent-weighted
     matmul: ps[r,c] = sum over covering words of 2^rank (per the row's
     band); ranks are distinct per band so the f32 exponent of the sum is
     exactly rank_max, i.e. widx = (bits >> 23) - 1 (-1 where uncovered).
  5. Paint: out[d, p] = ctab[band(p)][widx[p], d] via one-hot matmul in
     fp16, one 128-word chunk per 512-pixel slice (slices never straddle
     bands: 3072 % 512 == 0).
"""

import sys

import numpy as np

try:
    import concourse.bass as bass
except ImportError:  # grading env fallback
    sys.path.insert(0, "/opt/trn_rl_repo")
    import concourse.bass as bass

from concourse import bacc
import concourse.tile as tile
from concourse import mybir
from concourse.bass_utils import run_bass_kernel_spmd
from contextlib import ExitStack

P = 128
S, D = 512, 768
R, C, STRIDE = 128, 96, 8
T = S // P            # token tiles
NW = 256              # max word ids (each word spans >=2 tokens)
WT = NW // P          # word chunks for the global table (2)
NB = 4                # row bands (32 rows each)
BROWS = R // NB       # 32
BPIX = BROWS * C      # 3072 pixels per band
NPIX = R * C          # 12288
PG = 2048             # pixels per paint group
NG = NPIX // PG
DT = D // P
NH = PG // 512        # matmul column-slices per psum tile

F32 = mybir.dt.float32
F16 = mybir.dt.float16
BF16 = mybir.dt.bfloat16
I32 = mybir.dt.int32
OP = mybir.AluOpType

_last_results = None


def _const_blocks():
    """Host-precomputed constants, embedded in the NEFF (input-independent)."""
    p = np.arange(P, dtype=np.float32)[:, None]
    iota_r = np.broadcast_to(np.arange(R, dtype=np.float32), (P, R))
    iota_c = np.broadcast_to(np.arange(C, dtype=np.float32), (P, C))
    iota_w = np.broadcast_to(np.arange(NW, dtype=np.float32) - 1.0, (P, NW))
    iota_wp = np.broadcast_to(p, (P, 1)).copy()                 # palette ids
    bands_lo = np.broadcast_to(np.arange(NB, dtype=np.float32) * BROWS, (P, NB))
    bands_hi = bands_lo + BROWS
    rowband = np.broadcast_to(np.arange(R, dtype=np.float32) // BROWS, (P, R))
    cf32 = np.concatenate([iota_r, iota_c, iota_w, iota_wp,
                           bands_lo, bands_hi, rowband], axis=1)
    ii = np.arange(P)
    tri = (ii[:, None] <= ii[None, :]).astype(np.float16)  # [j, i] = j <= i
    ones = np.ones((P, P), dtype=np.float16)
    cf16 = np.concatenate([tri, ones], axis=1)
    return np.ascontiguousarray(cf32), np.ascontiguousarray(cf16)


def _build():
    nc = bacc.Bacc(None, target_bir_lowering=False)
    emb_ext = nc.declare_dram_parameter("emb", [S, D], F32, isOutput=False)
    coors_ext = nc.declare_dram_parameter("coors", [S, 4], I32, isOutput=False)
    mask_ext = nc.declare_dram_parameter("mask", [S, 1], I32, isOutput=False)
    out_ext = nc.declare_dram_parameter("out", [D, NPIX], F16, isOutput=True)
    widx_dram = nc.dram_tensor("widx_scratch", [P, C], F16)
    cf32_np, cf16_np = _const_blocks()
    cf32_ext = nc.inline_tensor(cf32_np, "cons_f32")
    cf16_ext = nc.inline_tensor(cf16_np, "cons_f16")

    with tile.TileContext(nc) as tc, ExitStack() as ctx:
        sing = ctx.enter_context(tc.tile_pool(name="sing", bufs=1))

        # ---- const + input loads, split across both HWDGE queues ----
        cf16 = sing.tile([P, 2 * P], F16, tag="cf16")
        nc.sync.dma_start(out=cf16[:], in_=cf16_ext[:])
        mask_all = sing.tile([P, T], I32, tag="mask_all")
        nc.scalar.dma_start(
            out=mask_all[:].rearrange("p (t o) -> p t o", t=T),
            in_=mask_ext[:].rearrange("(t p) o -> p t o", t=T))
        NC32 = R + C + NW + 1 + NB + NB + R
        cf32 = sing.tile([P, NC32], F32, tag="cf32")
        nc.scalar.dma_start(out=cf32[:], in_=cf32_ext[:])
        off = 0
        iota_r = cf32[:, off:off + R]; off += R
        iota_c = cf32[:, off:off + C]; off += C
        iotaW = cf32[:, off:off + NW]; off += NW
        iotawp = cf32[:, off:off + 1]; off += 1
        bandsLo = cf32[:, off:off + NB]; off += NB
        bandsHi = cf32[:, off:off + NB]; off += NB
        rowband = cf32[:, off:off + R]; off += R
        iota4 = iota_r[:, 0:NB]
        tri = cf16[:, 0:P]
        ones16 = cf16[:, P:2 * P]

        coors_all = sing.tile([P, 4 * T], I32, tag="coors_all")
        coors_re = coors_ext[:].rearrange("(t p) c -> p t c", t=T)
        nc.sync.dma_start(
            out=coors_all[:].rearrange("p (t c) -> p t c", t=T),
            in_=coors_re)
        coorsm1_all = sing.tile([P, 4 * T], I32, tag="coorsm1_all")
        nc.vector.memset(coorsm1_all[0:1, 0:4], -1)
        nc.sync.dma_start(out=coorsm1_all[1:P, 0:4],
                          in_=coors_ext[0:P - 1, :])
        nc.sync.dma_start(out=coorsm1_all[:, 4:8],
                          in_=coors_ext[P - 1:2 * P - 1, :])
        nc.scalar.dma_start(out=coorsm1_all[:, 8:12],
                            in_=coors_ext[2 * P - 1:3 * P - 1, :])
        nc.scalar.dma_start(out=coorsm1_all[:, 12:16],
                            in_=coors_ext[3 * P - 1:4 * P - 1, :])
        embT = sing.tile([P, T * (D + 1)], F16, tag="embT")
        embT3 = embT[:].rearrange("p (t e) -> p t e", e=D + 1)
        embext = [embT[:, t * (D + 1):(t + 1) * (D + 1)] for t in range(T)]

        # ---- batched per-token quantities ----
        mf = sing.tile([P, T], F32, tag="maskf")
        nc.vector.tensor_copy(mf[:], mask_all[:])
        invm4 = sing.tile([P, T], F16, tag="invm4")
        nc.vector.tensor_scalar(out=invm4[:], in0=mf[:], scalar1=1.0,
                                scalar2=-1.0, op0=OP.subtract, op1=OP.mult)
        cf = sing.tile([P, 4 * T], F32, tag="coorsf")
        nc.vector.tensor_copy(cf[:], coors_all[:])
        cm1f = sing.tile([P, 4 * T], F32, tag="coorsm1f")
        nc.vector.tensor_copy(cm1f[:], coorsm1_all[:])
        nc.gpsimd.memset(embT3[:, :, D:D + 1], 1.0)
        nc.vector.tensor_copy(
            embT3[:, :, 0:1],
            cm1f[:].rearrange("p (t c) -> p t c", t=T)[:, :, 0:1])
        nc.gpsimd.dma_start(
            out=embT3[:, :, 0:D],
            in_=emb_ext[:].rearrange("(t p) d -> p t d", t=T))
        eq16 = sing.tile([P, 4 * T], F32, tag="eq16")
        nc.vector.tensor_tensor(eq16[:], cf[:], cm1f[:], OP.is_equal)
        same4 = sing.tile([P, T], F32, tag="same4")
        nc.vector.tensor_reduce(same4[:],
                                eq16[:].rearrange("p (t c) -> p t c", t=T),
                                mybir.AxisListType.X, OP.min)
        wci = sing.tile([P, 4 * T], I32, tag="wci")
        nc.vector.tensor_scalar(out=wci[:], in0=coors_all[:], scalar1=3,
                                scalar2=None, op0=OP.arith_shift_right)
        wcf = sing.tile([P, 4 * T], F32, tag="wcf")
        nc.vector.tensor_copy(wcf[:], wci[:])
        wcf3 = wcf[:].rearrange("p (t c) -> p t c", t=T)
        wci3 = wci[:].rearrange("p (t c) -> p t c", t=T)

        # ---- valid / seg cumsums + per-band token ranks ----
        valid4 = sing.tile([P, T], F32, tag="valid4")
        nw4 = sing.tile([P, T], F32, tag="nw4")
        nwb4 = sing.tile([P, T], F16, tag="nwb4")
        seg4 = sing.tile([P, T], F32, tag="seg4")
        rank0 = sing.tile([P, T], F32, tag="rank0")
        rank1 = sing.tile([P, T], F32, tag="rank1")
        b0f = sing.tile([P, T], F32, tag="b0f")
        with tc.tile_pool(name="psA", bufs=1, space="PSUM") as psA:
            vps = psA.tile([P, T], F32, tag="vps", name="vps")
            for mt in range(T):
                for kc in range(mt + 1):
                    nc.tensor.matmul(out=vps[:, mt:mt + 1],
                                     lhsT=(tri if kc == mt else ones16),
                                     rhs=invm4[:, kc:kc + 1],
                                     start=(kc == 0), stop=(kc == mt))
            nc.vector.tensor_scalar(out=valid4[:], in0=vps[:],
                                    scalar1=0.5, scalar2=None, op0=OP.is_lt)
            nc.vector.scalar_tensor_tensor(out=nw4[:], in0=same4[:], scalar=0.5,
                                           in1=valid4[:], op0=OP.is_lt,
                                           op1=OP.mult)
            nc.vector.tensor_copy(nwb4[:], nw4[:])

            # coverage masks only need wcf -- keep DVE busy during cumsums
            rowcov, colcov = [], []
            for t in range(T):
                y0, y1 = wcf[:, 4 * t + 1:4 * t + 2], wcf[:, 4 * t + 3:4 * t + 4]
                x0, x1 = wcf[:, 4 * t + 0:4 * t + 1], wcf[:, 4 * t + 2:4 * t + 3]
                tge = sing.tile([P, R], F32, tag="tge")
                nc.vector.tensor_scalar(out=tge[:], in0=iota_r, scalar1=y0,
                                        scalar2=None, op0=OP.is_ge)
                rc = sing.tile([P, R], BF16, tag=f"rowcov{t}")
                nc.vector.scalar_tensor_tensor(out=rc[:], in0=iota_r, scalar=y1,
                                               in1=tge[:], op0=OP.is_lt,
                                               op1=OP.mult)
                rowcov.append(rc)
                cge = sing.tile([P, C], F32, tag="cge")
                nc.vector.tensor_scalar(out=cge[:], in0=iota_c, scalar1=x0,
                                        scalar2=None, op0=OP.is_ge)
                ccv = sing.tile([P, C], BF16, tag=f"colcov{t}")
                nc.vector.scalar_tensor_tensor(out=ccv[:], in0=iota_c, scalar=x1,
                                               in1=cge[:], op0=OP.is_lt,
                                               op1=OP.mult)
                colcov.append(ccv)

            sps = psA.tile([P, T], F32, tag="sps", name="sps")
            for mt in range(T):
                for kc in range(mt + 1):
                    nc.tensor.matmul(out=sps[:, mt:mt + 1],
                                     lhsT=(tri if kc == mt else ones16),
                                     rhs=nwb4[:, kc:kc + 1],
                                     start=(kc == 0), stop=(kc == mt))
            nc.vector.tensor_scalar(out=seg4[:], in0=sps[:],
                                    scalar1=1.0, scalar2=None, op0=OP.subtract)

            # per-token band hits: token box rows [y0g, y1g) vs 32-row bands
            y0gv = wcf3[:, :, 1:2]
            y1gv = wcf3[:, :, 3:4]
            hit = sing.tile([P, T * NB], F32, tag="hit")
            hitB = sing.tile([P, T * NB], F32, tag="hitB")
            hit3 = hit[:].rearrange("p (t b) -> p t b", t=T)
            hitB3 = hitB[:].rearrange("p (t b) -> p t b", t=T)
            nc.vector.tensor_tensor(hit3,
                                    y0gv.broadcast_to([P, T, NB]),
                                    bandsHi.unsqueeze(1).broadcast_to([P, T, NB]),
                                    OP.is_lt)
            nc.vector.tensor_tensor(hitB3,
                                    y1gv.broadcast_to([P, T, NB]),
                                    bandsLo.unsqueeze(1).broadcast_to([P, T, NB]),
                                    OP.is_gt)
            nc.vector.tensor_tensor(hit[:], hit[:], hitB[:], OP.mult)
            nwhit16 = sing.tile([P, T * NB], F16, tag="nwhit16")
            nc.vector.tensor_tensor(
                nwhit16[:].rearrange("p (t b) -> p t b", t=T),
                hit3, nw4[:].unsqueeze(2).broadcast_to([P, T, NB]), OP.mult)

            rps = psA.tile([P, T * NB], F32, tag="rps", name="rps")
            for mt in range(T):
                for kc in range(mt + 1):
                    nc.tensor.matmul(out=rps[:, mt * NB:(mt + 1) * NB],
                                     lhsT=(tri if kc == mt else ones16),
                                     rhs=nwhit16[:, kc * NB:(kc + 1) * NB],
                                     start=(kc == 0), stop=(kc == mt))

            # token's own band b0 = y0g >> 5 (box touches b0, maybe b0+1)
            b0i = sing.tile([P, T], I32, tag="b0i")
            nc.vector.tensor_scalar(
                out=b0i[:].rearrange("p (t o) -> p t o", o=1),
                in0=wci3[:, :, 1:2], scalar1=5,
                scalar2=None, op0=OP.arith_shift_right)
            nc.vector.tensor_copy(b0f[:], b0i[:])
            b0p1 = sing.tile([P, T], F32, tag="b0p1")
            nc.vector.tensor_scalar(out=b0p1[:], in0=b0f[:], scalar1=1.0,
                                    scalar2=None, op0=OP.add)
            oneh = sing.tile([P, T * NB], F32, tag="oneh")
            oneh3 = oneh[:].rearrange("p (t b) -> p t b", t=T)
            for bsel, rk in ((b0f, rank0), (b0p1, rank1)):
                nc.vector.tensor_tensor(
                    oneh3, iota4.unsqueeze(1).broadcast_to([P, T, NB]),
                    bsel[:].unsqueeze(2).broadcast_to([P, T, NB]), OP.is_equal)
                nc.vector.tensor_tensor(oneh[:], oneh[:], rps[:], OP.mult)
                nc.vector.tensor_reduce(rk[:], oneh3, mybir.AxisListType.X,
                                        OP.add)

        # scan weights: 2^rank per touched band (rank = pal_id + 1, distinct
        # within a band, so exponent of the pixel-sum = max rank)
        cw0 = sing.tile([P, T], F32, tag="cw0")
        cw1 = sing.tile([P, T], F32, tag="cw1")
        rbits = sing.tile([P, T], I32, tag="rbits")
        for rk, cw in ((rank0, cw0), (rank1, cw1)):
            nc.vector.tensor_copy(rbits[:], rk[:])
            nc.vector.tensor_scalar(out=rbits[:], in0=rbits[:], scalar1=23,
                                    scalar2=None, op0=OP.logical_shift_left)
            nc.vector.tensor_tensor(cw[:], rbits[:].bitcast(F32), nw4[:],
                                    OP.mult)
        rc0s, rc1s, rhs0s, rhs1s = [], [], [], []
        for t in range(T):
            rc0 = sing.tile([P, R], BF16, tag=f"rc0_{t}")
            nc.vector.scalar_tensor_tensor(out=rc0[:], in0=rowband,
                                           scalar=b0f[:, t:t + 1],
                                           in1=rowcov[t][:], op0=OP.is_equal,
                                           op1=OP.mult)
            rc1 = sing.tile([P, R], BF16, tag=f"rc1_{t}")
            nc.vector.tensor_tensor(rc1[:], rowcov[t][:], rc0[:], OP.subtract)
            r0 = sing.tile([P, C], BF16, tag=f"rhs0_{t}")
            nc.vector.tensor_scalar(out=r0[:], in0=colcov[t][:],
                                    scalar1=cw0[:, t:t + 1], scalar2=None,
                                    op0=OP.mult)
            r1 = sing.tile([P, C], BF16, tag=f"rhs1_{t}")
            nc.vector.tensor_scalar(out=r1[:], in0=colcov[t][:],
                                    scalar1=cw1[:, t:t + 1], scalar2=None,
                                    op0=OP.mult)
            rc0s.append(rc0); rc1s.append(rc1)
            rhs0s.append(r0); rhs1s.append(r1)

        # word-level one-hots: Onw for word boxes, Opr for the mean table
        seg4m1 = sing.tile([P, T], F32, tag="seg4m1")
        nc.vector.tensor_scalar(out=seg4m1[:], in0=seg4[:], scalar1=1.0,
                                scalar2=None, op0=OP.subtract)
        Onw, Opr = [], []
        for t in range(T):
            o = sing.tile([P, NW], F16, tag=f"onw{t}")
            nc.vector.tensor_scalar(out=o[:], in0=iotaW,
                                    scalar1=seg4m1[:, t:t + 1],
                                    scalar2=nw4[:, t:t + 1],
                                    op0=OP.is_equal, op1=OP.mult)
            Onw.append(o)
            o2 = sing.tile([P, NW], F16, tag=f"op{t}")
            nc.vector.tensor_scalar(out=o2[:], in0=iotaW,
                                    scalar1=seg4[:, t:t + 1],
                                    scalar2=valid4[:, t:t + 1],
                                    op0=OP.is_equal, op1=OP.mult)
            Opr.append(o2)
        ybox16 = sing.tile([P, T * 2], F16, tag="ybox16")
        ybox3 = ybox16[:].rearrange("p (t c) -> p t c", t=T)
        nc.vector.tensor_copy(ybox3[:, :, 0:1], wcf3[:, :, 1:2])
        nc.vector.tensor_copy(ybox3[:, :, 1:2], wcf3[:, :, 3:4])

        widx16 = sing.tile([P, C], F16, tag="widx16")
        widx_i = sing.tile([P, C], I32, tag="widx_i")
        table16 = []
        Pw = [[None] * WT for _ in range(NB)]
        with tc.tile_pool(name="psC", bufs=1, space="PSUM") as psC:
            # pixel scan: one accumulated stage over both touched bands
            ps1 = psC.tile([P, C], F32, tag="ps1")
            nmm = 2 * T
            k = 0
            for t in range(T):
                for rc, rh in ((rc0s[t], rhs0s[t]), (rc1s[t], rhs1s[t])):
                    nc.tensor.matmul(out=ps1[:], lhsT=rc[:], rhs=rh[:],
                                     start=(k == 0), stop=(k == nmm - 1))
                    k += 1
            nc.vector.tensor_scalar(out=widx_i[:], in0=ps1[:].bitcast(I32),
                                    scalar1=23, scalar2=None,
                                    op0=OP.logical_shift_right)
            nc.vector.tensor_scalar(out=widx_i[:], in0=widx_i[:], scalar1=1,
                                    scalar2=None, op0=OP.subtract)
            nc.vector.tensor_copy(widx16[:], widx_i[:])
            nc.sync.dma_start(out=widx_dram[:], in_=widx16[:])
            widx_flat = widx_dram[:].rearrange("p c -> (p c)")
            widx_g = []
            for g in range(NG):
                wg = sing.tile([P, PG], F16, tag=f"widx_g{g}")
                nc.sync.dma_start(
                    out=wg[:],
                    in_=widx_flat[g * PG:(g + 1) * PG].partition_broadcast(P))
                widx_g.append(wg)

            # word boxes: Wy[w, 2ch:2ch+2] = (y0g, y1g) of word w (chunk ch)
            Wy = psC.tile([P, 2 * WT], F32, tag="Wy")
            for ch in range(WT):
                for t in range(T):
                    nc.tensor.matmul(out=Wy[:, 2 * ch:2 * ch + 2],
                                     lhsT=Onw[t][:, ch * P:(ch + 1) * P],
                                     rhs=ybox16[:, 2 * t:2 * t + 2],
                                     start=(t == 0), stop=(t == T - 1))
            hitwf = sing.tile([P, WT * NB], F32, tag="hitwf")
            hitwB = sing.tile([P, WT * NB], F32, tag="hitwB")
            for ch in range(WT):
                sl = slice(ch * NB, (ch + 1) * NB)
                nc.vector.tensor_tensor(
                    hitwf[:, sl],
                    Wy[:, 2 * ch:2 * ch + 1].broadcast_to([P, NB]),
                    bandsHi, OP.is_lt)
                nc.vector.tensor_tensor(
                    hitwB[:, sl],
                    Wy[:, 2 * ch + 1:2 * ch + 2].broadcast_to([P, NB]),
                    bandsLo, OP.is_gt)
            nc.vector.tensor_tensor(hitwf[:], hitwf[:], hitwB[:], OP.mult)
            hitw16 = sing.tile([P, WT * NB], F16, tag="hitw16")
            nc.vector.tensor_copy(hitw16[:], hitwf[:])

            wrps = psC.tile([P, WT * NB], F32, tag="wrps")
            for mc in range(WT):
                for kc in range(mc + 1):
                    nc.tensor.matmul(out=wrps[:, mc * NB:(mc + 1) * NB],
                                     lhsT=(tri if kc == mc else ones16),
                                     rhs=hitw16[:, kc * NB:(kc + 1) * NB],
                                     start=(kc == 0), stop=(kc == mc))
            palwf = sing.tile([P, WT * NB], F32, tag="palwf")
            nc.vector.tensor_scalar(out=palwf[:], in0=wrps[:], scalar1=1.0,
                                    scalar2=None, op0=OP.subtract)
            for b in range(NB):
                for ch in range(WT):
                    pw = sing.tile([P, P], F16, tag=f"pw{b}_{ch}")
                    col = ch * NB + b
                    nc.vector.tensor_scalar(out=pw[:], in0=iota_r,
                                            scalar1=palwf[:, col:col + 1],
                                            scalar2=hitwf[:, col:col + 1],
                                            op0=OP.is_equal, op1=OP.mult)
                    Pw[b][ch] = pw

            # global word mean table (shifted): table[w] = mean(word w-1)
            with tc.tile_pool(name="psD", bufs=2, space="PSUM") as psD:
                for wt in range(WT):
                    ptab = psD.tile([P, 1024], F32, tag="ptab", name=f"ptab{wt}")
                    for kc in range(T):
                        lhs = Opr[kc][:, wt * P:(wt + 1) * P]
                        nc.tensor.matmul(out=ptab[:, 0:512], lhsT=lhs,
                                         rhs=embext[kc][:, 0:512],
                                         start=(kc == 0), stop=(kc == T - 1))
                        nc.tensor.matmul(out=ptab[:, 512:D + 1], lhsT=lhs,
                                         rhs=embext[kc][:, 512:D + 1],
                                         start=(kc == 0), stop=(kc == T - 1))
                    rec = sing.tile([P, 1], F32, tag="rec")
                    nc.vector.tensor_scalar(out=rec[:], in0=ptab[:, D:D + 1],
                                            scalar1=1.0, scalar2=None,
                                            op0=OP.max)
                    recr = sing.tile([P, 1], F32, tag="recr")
                    nc.vector.reciprocal(recr[:], rec[:])
                    tb = sing.tile([P, D], F16, tag=f"table{wt}")
                    nc.scalar.mul(out=tb[:], in_=ptab[:, 0:D], mul=recr[:, 0:1])
                    table16.append(tb)

        # compact per-band tables: ctab[b] = Pw[b]^T @ table
        ctab16 = []
        with tc.tile_pool(name="psE", bufs=2, space="PSUM") as psE:
            for b in range(NB):
                cps = psE.tile([P, D], F32, tag="cps", name=f"cps{b}")
                for ch in range(WT):
                    nc.tensor.matmul(out=cps[:, 0:512], lhsT=Pw[b][ch][:],
                                     rhs=table16[ch][:, 0:512],
                                     start=(ch == 0), stop=(ch == WT - 1))
                    nc.tensor.matmul(out=cps[:, 512:D], lhsT=Pw[b][ch][:],
                                     rhs=table16[ch][:, 512:D],
                                     start=(ch == 0), stop=(ch == WT - 1))
                ct = sing.tile([P, D], F16, tag=f"ctab{b}")
                if b % 2 == 0:
                    nc.vector.tensor_copy(ct[:], cps[:])
                else:
                    nc.scalar.copy(out=ct[:], in_=cps[:])
                ctab16.append(ct)

        # ---- paint: out[d, p] = ctab[band(p)][widx[p], d] ----
        with tc.tile_pool(name="oh", bufs=2) as ohp, \
             tc.tile_pool(name="stage", bufs=6) as stp, \
             tc.tile_pool(name="pp", bufs=2, space="PSUM") as ppp:
            for g in range(NG):
                gs = slice(g * PG, (g + 1) * PG)
                oh = ohp.tile([P, PG], F16, tag="oh", name="oh")
                nc.vector.tensor_scalar(out=oh[:], in0=widx_g[g][:],
                                        scalar1=iotawp[:, 0:1],
                                        scalar2=None, op0=OP.is_equal)
                for dt in range(DT):
                    stage = stp.tile([P, PG], F16, tag="stage", name="stage")
                    dsl = slice(dt * P, (dt + 1) * P)
                    # two independent half-width PSUM tiles per dt: each copy
                    # (PSUM-read bound) waits only on its own 2 matmuls, and
                    # 4 half-units are in flight in the same 8-bank budget
                    for half in range(2):
                        pp = ppp.tile([P, PG // 2], F32, tag=f"pp{half}",
                                      name=f"pp{half}")
                        for s3h in range(NH // 2):
                            s3 = half * (NH // 2) + s3h
                            band = (g * PG + s3 * 512) // BPIX
                            nc.tensor.matmul(
                                out=pp[:, s3h * 512:(s3h + 1) * 512],
                                lhsT=ctab16[band][:, dsl],
                                rhs=oh[:, s3 * 512:(s3 + 1) * 512],
                                start=True, stop=True)
                        hsl = slice(half * (PG // 2), (half + 1) * (PG // 2))
                        if half == 0:
                            nc.vector.tensor_copy(stage[:, hsl], pp[:])
                        else:
                            nc.scalar.copy(out=stage[:, hsl], in_=pp[:])
                    if dt % 2 == 0:
                        nc.sync.dma_start(out=out_ext[dt * P:(dt + 1) * P, gs],
                                          in_=stage[:])
                    else:
                        nc.scalar.dma_start(out=out_ext[dt * P:(dt + 1) * P, gs],
                                            in_=stage[:])
    nc.compile()
    return nc


_nc_cache = None


def kernel(bert_embeddings, coors, mask, image_h=1024, image_w=768, stride=8):
    global _last_results, _nc_cache
    emb = np.ascontiguousarray(np.asarray(bert_embeddings, dtype=np.float32))
    co = np.ascontiguousarray(np.asarray(coors, dtype=np.int32))
    mk = np.ascontiguousarray(np.asarray(mask, dtype=np.int32))
    ih, iw, st = int(image_h), int(image_w), int(stride)
    B = emb.shape[0]
    assert (ih // st, iw // st) == (R, C) and st == STRIDE
    assert emb.shape == (B, S, D) and B == 8

    if _nc_cache is None:
        _nc_cache = _build()
    nc = _nc_cache

    in_maps = [{"emb": emb[b], "coors": co[b], "mask": mk[b].reshape(S, 1)}
               for b in range(B)]
    res = run_bass_kernel_spmd(nc, in_maps, core_ids=list(range(B)))
    _last_results = res
    out = np.stack([np.asarray(res.results[b]["out"]).reshape(D, R, C)
                    for b in range(B)])
    return out.astype(np.float32)
